# revision 1
# baseline (speedup 1.0000x reference)
"""GAT recommender (2-layer GAT + residual + dot scoring) on 8 Trainium2 cores.

Strategy (edge-parallel, dst-sharded):
  - Sort edges by dst; core k owns a contiguous node range (W windows of 128
    nodes) and all edges whose dst falls in it.
  - Dense phases are replicated (each core computes the full node tables);
    per-edge work is sharded.
  - Per window: dma_gather fetches packed rows [h | e_src] of the src nodes
    (5 int16-indexed banks), one-hot matrices built on-device expand e_dst
    and scatter-accumulate messages + attention sums into PSUM via PE
    matmuls, then normalize.
  - exp(att - M) uses the upper bound M = lrelu(max e_src + max e_dst); the
    softmax normalization cancels the difference vs the reference's global
    max up to the 1e-8 epsilon (relative effect ~1e-9).
  - Layer 2 tables are computed per-core on owned nodes and AllGathered;
    final embeddings AllGathered; scoring dots computed on-device.
"""

import numpy as np

try:
    import concourse.bacc as bacc
except ImportError:  # harness fresh-dir fallback
    import sys
    for p in ("/opt/trn_rl_repo", "/root/.axon_site/_ro/trn_rl_repo"):
        if p not in sys.path:
            sys.path.insert(0, p)
    import concourse.bacc as bacc

import concourse.mybir as mybir
import concourse.tile as tile
from concourse.bass import ds, IndirectOffsetOnAxis
from concourse.bass_isa import ReduceOp
from concourse.bass_utils import run_bass_kernel_spmd

P = 128
EMB = 64
H1 = 4          # layer-1 heads
NBANKS = 5
BANK = 30720    # int16-indexable bank rows (<= 32768)
CE1 = 384       # table1 row in bf16 units: [h1 256 bf16 | e1_src 4 f32 | pad] -> 768B
CE1U = 260      # scatter columns (msg 256 + att 4, bf16)
CE2 = 128       # table2 row in bf16 units: [h2 64 bf16 | e2_src,e2_dst f32] -> 256B
CE2U = 66       # dense-2 psum columns (h2 64 | e2_src | e2_dst, f32)
EPS = 1e-8
LRELU = 0.2

F32 = mybir.dt.float32
BF16 = mybir.dt.bfloat16
I32 = mybir.dt.int32
I16 = mybir.dt.int16
AF = mybir.ActivationFunctionType
OP = mybir.AluOpType


def _leaky_max(nc, pool, ap, tag):
    """in-place x <- max(x, LRELU*x)"""
    shape = [ap.shape[0], int(np.prod(ap.shape[1:]))]
    tmp = pool.tile(shape, F32, tag=tag)
    nc.vector.tensor_scalar_mul(out=tmp[:], in0=ap, scalar1=LRELU)
    nc.vector.tensor_tensor(out=ap, in0=ap, in1=tmp[:], op=OP.max)


DEBUG_L2 = False
# Exact per-window gather counts (skip-padded idx lists) are functionally
# verified in isolation but showed completion races in the full pipeline on
# HW; static full counts with valid-row padding are used instead.
STATIC_COUNTS = True
DEBUG_INTERNALS = False


def build_program(cfg):
    """Builds the SPMD Bass program. cfg: dict with W, Tb, NSTAR, NC_NODES,
    NTAB, BB (batch per core), CORES, unrolls."""
    W, Tb, NB = cfg["W"], cfg["Tb"], cfg["NB"]
    NSTAR, NCN, NTAB = cfg["NSTAR"], cfg["NC_NODES"], cfg["NTAB"]
    BB, CORES = cfg["BB"], cfg["CORES"]
    TT = NB * Tb              # tiles per window
    NIDXB = Tb * P            # idxs per bank gather call
    ICOLS = NB * Tb * 8       # int16 idx columns per window
    DT1 = NSTAR // P          # dense-1 tiles (all nodes)
    DTO = NCN // P            # dense tiles (own nodes)
    groups = [list(range(CORES))]

    nc = bacc.Bacc("TRN2", target_bir_lowering=False, debug=False,
                   num_devices=CORES)

    # ---------- inputs ----------
    xT_in = nc.dram_tensor("xT", [EMB, NSTAR], F32, kind="ExternalInput")
    xTo_in = nc.dram_tensor("xTo", [EMB, NCN], F32, kind="ExternalInput")
    xo_in = nc.dram_tensor("xo", [NCN, EMB], F32, kind="ExternalInput")
    W1p_in = nc.dram_tensor("W1p", [EMB, 264], F32, kind="ExternalInput")
    W2p_in = nc.dram_tensor("W2p", [4 * EMB, CE2U], F32, kind="ExternalInput")
    iota_in = nc.dram_tensor("iota", [P, P], F32, kind="ExternalInput")
    ident_in = nc.dram_tensor("ident", [P, P], F32, kind="ExternalInput")
    dstw_in = nc.dram_tensor("dstw", [W * P, TT], F32, kind="ExternalInput")
    idxw_in = nc.dram_tensor("idxw", [W * P, ICOLS], I16, kind="ExternalInput")
    cnts_in = nc.dram_tensor("cnts", [W, NB], I32, kind="ExternalInput")
    cnts2_in = nc.dram_tensor("cnts2", [W, NB], I32, kind="ExternalInput")
    uidx_in = nc.dram_tensor("uidx", [P, BB // P], I32, kind="ExternalInput")
    iidx_in = nc.dram_tensor("iidx", [P, BB // P], I32, kind="ExternalInput")

    # ---------- outputs / intermediates ----------
    out_t = nc.dram_tensor("out", [P, BB // P], F32, kind="ExternalOutput")
    TTd = NB * Tb
    if DEBUG_L2:
        dbg_edw = nc.dram_tensor("dbg_edw", [P, 1], F32, kind="ExternalOutput")
        dbg_att = nc.dram_tensor("dbg_att", [P, TTd], F32, kind="ExternalOutput")
        dbg_gb = nc.dram_tensor("dbg_gb", [P, TTd * CE2], F32, kind="ExternalOutput")
        dbg_win = nc.dram_tensor("dbg_win", [P, CE2U - 1], F32, kind="ExternalOutput")
        dbg_cnt = nc.dram_tensor("dbg_cnt", [1, NB], F32, kind="ExternalOutput")
    _ik = "ExternalOutput" if DEBUG_INTERNALS else "Internal"
    table1 = nc.dram_tensor("table1", [NTAB, CE1], BF16, kind="Internal")
    e1do = nc.dram_tensor("e1do", [NCN, H1], F32, kind=_ik)
    x2o = nc.dram_tensor("x2o", [NCN, 4 * EMB], F32, kind=_ik)
    t2own = nc.dram_tensor("t2own", [NCN, CE2], BF16, kind=_ik)
    t2full = nc.dram_tensor("t2full", [NTAB, CE2], BF16, kind="Internal",
                            addr_space="Shared")
    m2loc = nc.dram_tensor("m2loc", [1, 2], F32, kind="Internal")
    m2sh = nc.dram_tensor("m2sh", [1, 2], F32, kind="Internal",
                          addr_space="Shared")
    hown = nc.dram_tensor("hown", [NCN, EMB], F32, kind=_ik)
    hfull = nc.dram_tensor("hfull", [CORES * NCN, EMB], F32, kind="Internal",
                           addr_space="Shared")

    with tile.TileContext(nc) as tc:
        with (
            tc.tile_pool(name="const", bufs=1) as cp,
            tc.tile_pool(name="sb", bufs=2) as sb,
            tc.tile_pool(name="gb", bufs=1) as gbp,
            tc.tile_pool(name="sall", bufs=2) as sap,
            tc.tile_pool(name="ps", bufs=2, space="PSUM") as pp,
            tc.tile_pool(name="pst", bufs=3, space="PSUM") as pst,
            tc.tile_pool(name="pw", bufs=2, space="PSUM") as pw,
        ):
            iota = cp.tile([P, P], F32)
            nc.sync.dma_start(out=iota[:], in_=iota_in[:, :])
            ident = cp.tile([P, P], F32)
            nc.sync.dma_start(out=ident[:], in_=ident_in[:, :])
            identb = cp.tile([P, P], BF16)
            nc.vector.tensor_copy(out=identb[:], in_=ident[:])
            W1s = cp.tile([EMB, 264], F32)
            nc.sync.dma_start(out=W1s[:], in_=W1p_in[:, :])
            W2s = cp.tile([P, 2, CE2U], F32)
            nc.sync.dma_start(out=W2s[:, :, :],
                              in_=W2p_in[:, :].rearrange("(k p) c -> p k c", p=P))
            rmax1 = cp.tile([P, 8], F32)
            nc.vector.memset(rmax1[:], -3.0e38)
            rmax2 = cp.tile([P, 2], F32)
            nc.vector.memset(rmax2[:], -3.0e38)
            mneg1 = cp.tile([P, 1], F32)
            mneg2 = cp.tile([P, 1], F32)

            # ================= dense-1 A: full table1 + running max ========
            U = cfg["UN_D1"]
            with tc.For_i(0, DT1 // U) as i0:
                for k in range(U):
                    lt = sb.tile([EMB, P], F32, tag="d1l")
                    nc.sync.dma_start(out=lt[:],
                                      in_=xT_in[:, ds(i0 * (U * P) + k * P, P)])
                    hp = pp.tile([P, 264], F32, tag="gp")
                    nc.tensor.matmul(out=hp[:], lhsT=lt[:], rhs=W1s[:],
                                     start=True, stop=True)
                    hs = sb.tile([P, 264], BF16, tag="d1s")
                    nc.vector.tensor_copy(out=hs[:, 0:256], in_=hp[:, 0:256])
                    nc.vector.tensor_copy(out=hs[:, 256:264].bitcast(F32),
                                          in_=hp[:, 256:260])
                    nc.vector.tensor_tensor(out=rmax1[:], in0=rmax1[:],
                                            in1=hp[:, 256:264], op=OP.max)
                    nc.sync.dma_start(
                        out=table1[ds(i0 * (U * P) + k * P, P), 0:264],
                        in_=hs[:, 0:264])

            # ================= dense-1 B: own e1_dst ======================
            U = cfg["UN_D1B"]
            with tc.For_i(0, DTO // U) as i1:
                for k in range(U):
                    lt = sb.tile([EMB, P], F32, tag="d1bl")
                    nc.sync.dma_start(out=lt[:],
                                      in_=xTo_in[:, ds(i1 * (U * P) + k * P, P)])
                    ep = pp.tile([P, 264], F32, tag="gp")
                    nc.tensor.matmul(out=ep[:, 0:H1], lhsT=lt[:], rhs=W1s[:, 260:264],
                                     start=True, stop=True)
                    es = sb.tile([P, H1], F32, tag="d1bs")
                    nc.vector.tensor_copy(out=es[:], in_=ep[:, 0:H1])
                    nc.sync.dma_start(out=e1do[ds(i1 * (U * P) + k * P, P), :],
                                      in_=es[:])

            # ================= M1 bound ===================================
            rr1 = sb.tile([P, 8], F32, tag="rr1")
            nc.gpsimd.partition_all_reduce(rr1[:], rmax1[:], P, ReduceOp.max)
            ma = sb.tile([P, 1], F32, tag="ma")
            mb = sb.tile([P, 1], F32, tag="mb")
            nc.vector.reduce_max(ma[:], rr1[:, 0:4], axis=mybir.AxisListType.X)
            nc.vector.reduce_max(mb[:], rr1[:, 4:8], axis=mybir.AxisListType.X)
            nc.vector.tensor_tensor(out=ma[:], in0=ma[:], in1=mb[:], op=OP.add)
            _leaky_max(nc, sb, ma[:], "mlk")
            nc.vector.tensor_scalar_mul(out=mneg1[:], in0=ma[:], scalar1=-1.0)

            # zero the empty-group fallback rows that dense phases never
            # write (only fires when a bank base lies past NSTAR; full-size
            # configs have none)
            zrow = sb.tile([P, CE1], BF16, tag="zrow")
            nc.vector.memset(zrow[:], 0.0)
            for b in range(NB):
                if b * BANK >= NSTAR:
                    nc.sync.dma_start(out=table1[b * BANK:b * BANK + 1, :],
                                      in_=zrow[0:1, :])
                    nc.sync.dma_start(out=t2full[b * BANK:b * BANK + 1, :],
                                      in_=zrow[0:1, 0:CE2])

            # gather-count registers (per unrolled window slot, per bank)
            UW = cfg["UN_WIN"]
            cregs = [[nc.gpsimd.alloc_register(f"cnt{k}_{b}")
                      for b in range(NB)] for k in range(UW)]

            # ================= layer-1 edge windows =======================
            def gat_window(w, lay, regs, gbuf):
                """Emit one window of GAT message passing for layer `lay`.
                regs: per-bank gpsimd count registers; gbuf: persistent
                bf16 gather buffer (pre-zeroed once)."""
                CEa = CE1 if lay == 1 else CE2
                NH = H1 if lay == 1 else 1
                NF = EMB
                CEu = NF * NH + NH                     # scatter cols (bf16)
                tab = table1 if lay == 1 else t2full
                edt = e1do if lay == 1 else None
                mneg = mneg1 if lay == 1 else mneg2
                tg = f"l{lay}"

                dstf = sb.tile([P, TT], F32, tag=tg + "dst")
                nc.sync.dma_start(out=dstf[:], in_=dstw_in[ds(w * P, P), :])
                idxt = sb.tile([P, ICOLS], I16, tag=tg + "idx")
                nc.sync.dma_start(out=idxt[:], in_=idxw_in[ds(w * P, P), :])
                cntt = sb.tile([1, NB], I32, tag=tg + "cnt")
                nc.sync.dma_start(
                    out=cntt[:],
                    in_=(cnts_in if lay == 1 else cnts2_in)[ds(w, 1), :])
                nc.gpsimd.reg_load(regs, cntt[0:1, 0:NB])
                edw = sb.tile([P, NH], F32, tag=tg + "edw")
                if lay == 1:
                    nc.sync.dma_start(out=edw[:], in_=edt[ds(w * P, P), :])
                else:
                    nc.sync.dma_start(
                        out=edw[:],
                        in_=t2own[ds(w * P, P), 66:68].bitcast(F32))

                for b in range(NB):
                    nc.gpsimd.dma_gather(
                        out_ap=gbuf[:, b * Tb:(b + 1) * Tb, :],
                        in_ap=tab[b * BANK:(b + 1) * BANK, :],
                        idxs_ap=idxt[:, b * Tb * 8:(b + 1) * Tb * 8],
                        num_idxs=NIDXB,
                        num_idxs_reg=regs[b],
                        elem_size=CEa,
                    )

                S_all = sap.tile([P, TT, P], BF16, tag=tg + "S")
                edp = pp.tile([P, 264], F32, tag="gp")
                for j in range(TT):
                    nc.vector.tensor_tensor(
                        out=S_all[:, j, :], in0=iota[:],
                        in1=dstf[:, j:j + 1].to_broadcast([P, P]),
                        op=OP.is_equal)
                    stp = pst.tile([P, P], BF16, tag="stp")
                    nc.tensor.transpose(out=stp[:], in_=S_all[:, j, :],
                                        identity=identb[:])
                    st = sb.tile([P, P], F32, tag=tg + "st")
                    nc.scalar.copy(out=st[:], in_=stp[:])
                    nc.tensor.matmul(out=edp[:, j * NH:(j + 1) * NH],
                                     lhsT=st[:], rhs=edw[:, :],
                                     start=True, stop=True)

                att = sb.tile([P, TT * NH], F32, tag=tg + "att")
                nc.vector.tensor_tensor(
                    out=att[:].rearrange("p (t h) -> p t h", h=NH),
                    in0=gbuf[:, :, NF * NH:NF * NH + 2 * NH].bitcast(F32),
                    in1=edp[:, 0:TT * NH].rearrange("p (t h) -> p t h", h=NH),
                    op=OP.add)
                _leaky_max(nc, sb, att[:], tg + "alk")
                nc.scalar.activation(
                    gbuf[:, :, NF * NH:NF * NH + NH], att[:].rearrange(
                        "p (t h) -> p t h", h=NH),
                    AF.Exp, bias=mneg[:])
                nc.vector.tensor_tensor(
                    out=gbuf[:, :, 0:NF * NH].rearrange(
                        "p t (h f) -> p t h f", h=NH),
                    in0=gbuf[:, :, 0:NF * NH].rearrange(
                        "p t (h f) -> p t h f", h=NH),
                    in1=gbuf[:, :, NF * NH:NF * NH + NH].to_broadcast(
                        [P, TT, NH, NF]),
                    op=OP.mult)

                wps = pw.tile([P, CE1U], F32, tag="wp")
                for j in range(TT):
                    nc.tensor.matmul(out=wps[:, 0:CEu], lhsT=S_all[:, j, :],
                                     rhs=gbuf[:, j, 0:CEu],
                                     start=(j == 0), stop=(j == TT - 1))

                if DEBUG_L2 and lay == 2 and not isinstance(w, int):
                    nc.sync.dma_start(out=dbg_edw[:, :], in_=edw[:, 0:1])
                    nc.sync.dma_start(out=dbg_att[:, :], in_=att[:, 0:TT])
                    nc.sync.dma_start(
                        out=dbg_gb[:, :],
                        in_=gbuf[:, :, :].rearrange("p t c -> p (t c)"))
                    wcp = sb.tile([P, CEu], F32, tag="wcp")
                    nc.vector.tensor_copy(out=wcp[:], in_=wps[:, 0:CEu])
                    nc.sync.dma_start(out=dbg_win[:, :], in_=wcp[:])
                    ccp = sb.tile([1, NB], F32, tag="ccp")
                    nc.vector.tensor_copy(out=ccp[:], in_=cntt[0:1, :])
                    nc.sync.dma_start(out=dbg_cnt[:, :], in_=ccp[:])
                rec = sb.tile([P, NH], F32, tag=tg + "rec")
                nc.vector.tensor_scalar_add(out=rec[:],
                                            in0=wps[:, NF * NH:NF * NH + NH],
                                            scalar1=EPS)
                nc.vector.reciprocal(out=rec[:], in_=rec[:])
                onr = sb.tile([P, NF * NH], F32, tag=tg + "on")
                nc.vector.tensor_tensor(
                    out=onr[:].rearrange("p (h f) -> p h f", h=NH),
                    in0=wps[:, 0:NF * NH].rearrange("p (h f) -> p h f", h=NH),
                    in1=rec[:].to_broadcast([P, NH, NF]),
                    op=OP.mult)
                return onr

            U = cfg["UN_WIN"]

            def l1_tail(w, onr):
                # ELU -> x2
                neg = sb.tile([P, 4 * EMB], F32, tag="l1neg")
                nc.vector.tensor_scalar_min(out=neg[:], in0=onr[:],
                                            scalar1=0.0)
                nc.scalar.activation(neg[:], neg[:], AF.Exp)
                pos = sb.tile([P, 4 * EMB], F32, tag="l1pos")
                nc.vector.tensor_scalar_max(out=pos[:], in0=onr[:],
                                            scalar1=0.0)
                nc.vector.tensor_tensor(out=pos[:], in0=pos[:], in1=neg[:],
                                        op=OP.add)
                nc.vector.tensor_scalar_add(out=pos[:], in0=pos[:],
                                            scalar1=-1.0)
                nc.sync.dma_start(out=x2o[ds(w * P, P), :], in_=pos[:])

            g1bufs = [gbp.tile([P, TT, CE1], BF16, tag=f"g1_{i}", name=f"g1_{i}")
                      for i in range(2)]
            for g in g1bufs:
                nc.vector.memset(g[:, :, :], 0.0)
            for k in range(U):  # peeled (program warms both buffers)
                l1_tail(k, gat_window(k, 1, cregs[k], g1bufs[k % 2]))
            with tc.For_i(1, W // U) as w0:
                for k in range(U):
                    w = w0 * U + k
                    l1_tail(w, gat_window(w, 1, cregs[k], g1bufs[k % 2]))

            # ================= dense-2: own h2/e2 tables ==================
            U = cfg["UN_D2"]
            with tc.For_i(0, DTO // U) as i2:
                for k in range(U):
                    x2t = sb.tile([P, 4 * EMB], F32, tag="d2x")
                    nc.sync.dma_start(out=x2t[:],
                                      in_=x2o[ds(i2 * (U * P) + k * P, P), :])
                    h2p = pp.tile([P, 264], F32, tag="gp")
                    for c in range(2):
                        ttp = pst.tile([P, P], F32, tag="stp")
                        nc.tensor.transpose(out=ttp[:],
                                            in_=x2t[:, c * P:(c + 1) * P],
                                            identity=ident[:])
                        tts = sb.tile([P, P], F32, tag="d2ts")
                        nc.vector.tensor_copy(out=tts[:], in_=ttp[:])
                        nc.tensor.matmul(out=h2p[:, 0:CE2U], lhsT=tts[:],
                                         rhs=W2s[:, c, :],
                                         start=(c == 0), stop=(c == 1))
                    h2s = sb.tile([P, CE2], BF16, tag="d2s")
                    nc.vector.tensor_copy(out=h2s[:, 0:64], in_=h2p[:, 0:64])
                    nc.vector.tensor_copy(out=h2s[:, 64:68].bitcast(F32),
                                          in_=h2p[:, 64:66])
                    nc.vector.tensor_tensor(out=rmax2[:], in0=rmax2[:],
                                            in1=h2p[:, 64:66], op=OP.max)
                    nc.sync.dma_start(
                        out=t2own[ds(i2 * (U * P) + k * P, P), 0:68],
                        in_=h2s[:, 0:68])

            # ================= M2 bound (allreduce) + table2 allgather ====
            rr2 = sb.tile([P, 2], F32, tag="rr2")
            nc.gpsimd.partition_all_reduce(rr2[:], rmax2[:], P, ReduceOp.max)
            nc.sync.dma_start(out=m2loc[:, :], in_=rr2[0:1, :])
            nc.gpsimd.collective_compute(
                "AllReduce", OP.max, replica_groups=groups,
                ins=[m2loc[:, :]], outs=[m2sh[:, :]])
            m2t = sb.tile([P, 2], F32, tag="m2t")
            nc.sync.dma_start(out=m2t[:], in_=m2sh[:, :].to_broadcast([P, 2]))
            nc.vector.tensor_tensor(out=m2t[:, 0:1], in0=m2t[:, 0:1],
                                    in1=m2t[:, 1:2], op=OP.add)
            _leaky_max(nc, sb, m2t[:, 0:1], "m2lk")
            nc.vector.tensor_scalar_mul(out=mneg2[:], in0=m2t[:, 0:1],
                                        scalar1=-1.0)

            nc.gpsimd.collective_compute(
                "AllGather", OP.bypass, replica_groups=groups,
                ins=[t2own[:, :]], outs=[t2full[0:CORES * NCN, :]])

            # ================= layer-2 edge windows =======================
            U = cfg["UN_WIN"]

            def l2_tail(w, onr):
                xot = sb.tile([P, EMB], F32, tag="l2xo")
                nc.sync.dma_start(out=xot[:], in_=xo_in[ds(w * P, P), :])
                nc.vector.tensor_tensor(out=xot[:], in0=xot[:], in1=onr[:],
                                        op=OP.add)
                nc.sync.dma_start(out=hown[ds(w * P, P), :], in_=xot[:])

            g2bufs = [gbp.tile([P, TT, CE2], BF16, tag=f"g2_{i}", name=f"g2_{i}")
                      for i in range(2)]
            for g in g2bufs:
                nc.vector.memset(g[:, :, :], 0.0)
            for k in range(U):
                l2_tail(k, gat_window(k, 2, cregs[k], g2bufs[k % 2]))
            with tc.For_i(1, W // U) as w1:
                for k in range(U):
                    w = w1 * U + k
                    l2_tail(w, gat_window(w, 2, cregs[k], g2bufs[k % 2]))

            # ================= final embeddings allgather + dots ==========
            nc.gpsimd.collective_compute(
                "AllGather", OP.bypass, replica_groups=groups,
                ins=[hown[:, :]], outs=[hfull[:, :]])

            uix = sb.tile([P, BB // P], I32, tag="uix")
            nc.sync.dma_start(out=uix[:], in_=uidx_in[:, :])
            iix = sb.tile([P, BB // P], I32, tag="iix")
            nc.sync.dma_start(out=iix[:], in_=iidx_in[:, :])
            ubuf = sb.tile([P, BB // P, EMB], F32, tag="ubuf")
            ibuf = sb.tile([P, BB // P, EMB], F32, tag="ibuf")
            for j in range(BB // P):
                nc.gpsimd.indirect_dma_start(
                    out=ubuf[:, j, :], out_offset=None, in_=hfull[:, :],
                    in_offset=IndirectOffsetOnAxis(ap=uix[:, j:j + 1], axis=0))
                nc.gpsimd.indirect_dma_start(
                    out=ibuf[:, j, :], out_offset=None, in_=hfull[:, :],
                    in_offset=IndirectOffsetOnAxis(ap=iix[:, j:j + 1], axis=0))
            nc.vector.tensor_tensor(
                out=ubuf[:, :, :], in0=ubuf[:, :, :], in1=ibuf[:, :, :],
                op=OP.mult)
            dots = sb.tile([P, BB // P], F32, tag="dots")
            nc.vector.reduce_sum(dots[:], ubuf[:, :, :],
                                 axis=mybir.AxisListType.X)
            nc.sync.dma_start(out=out_t[:, :], in_=dots[:])

    nc.compile()
    return nc


def prepare_inputs(user_table, item_table, W1, a1, W2, a2, edge_index,
                   user_ids, item_ids, cfg):
    W, CORES = cfg["W"], cfg["CORES"]
    NSTAR, NCN, BB = cfg["NSTAR"], cfg["NC_NODES"], cfg["BB"]
    NU = user_table.shape[0]
    N = NU + item_table.shape[0]

    x = np.concatenate([np.asarray(user_table, np.float32),
                        np.asarray(item_table, np.float32)], axis=0)
    xpad = np.zeros((NSTAR, EMB), np.float32)
    xpad[:N] = x
    xT = np.ascontiguousarray(xpad.T)

    W1 = np.asarray(W1, np.float32)
    a1 = np.asarray(a1, np.float32)
    W2 = np.asarray(W2, np.float32)
    a2 = np.asarray(a2, np.float32)
    A1l = np.stack([W1[:, h * EMB:(h + 1) * EMB] @ a1[h, :EMB]
                    for h in range(H1)], axis=1)
    A1r = np.stack([W1[:, h * EMB:(h + 1) * EMB] @ a1[h, EMB:]
                    for h in range(H1)], axis=1)
    W1p = np.concatenate([W1, A1l, A1r], axis=1)
    w2l = W2 @ a2[0, :EMB]
    w2r = W2 @ a2[0, EMB:]
    W2p = np.concatenate([W2, w2l[:, None], w2r[:, None]], axis=1)

    src = np.asarray(edge_index[0]).astype(np.int64)
    dst = np.asarray(edge_index[1]).astype(np.int64)
    NB = cfg["NB"]
    NWG = CORES * W
    key = (dst // P) * NB + (src // BANK)
    order = np.argsort(key, kind="stable")
    src_g, dst_g, key_g = src[order], dst[order], key[order]
    cnt = np.bincount(key_g, minlength=NWG * NB)
    Tb = cfg["Tb"]
    assert cnt.max() <= Tb * P, f"Tb={Tb} too small for {cnt.max()}"
    NIDXB = Tb * P
    gstart = np.concatenate([[0], np.cumsum(cnt)])[:-1]
    off = np.arange(len(src_g)) - gstart[key_g]
    slot = key_g * NIDXB + off
    bankidx = (src_g - (src_g // BANK) * BANK).astype(np.int16)

    def idx_layout(flat):
        A = flat.reshape(NWG, NB, Tb * 8, 16)
        A = np.transpose(A, (0, 1, 3, 2))                  # [wg, b, 16, cols]
        A = np.tile(A, (1, 1, 8, 1))                       # [wg, b, 128, cols]
        return np.ascontiguousarray(
            np.transpose(A, (0, 2, 1, 3)).reshape(NWG, P, NB * Tb * 8))

    # skip-padded (-1 trailing = no DMA); empty groups get one row 0
    if STATIC_COUNTS:
        idx_flatn = np.zeros(NWG * NB * NIDXB, np.int16)
        idx_flatn[slot] = bankidx
        cnts = np.full((NWG, NB), NIDXB, np.int32)
    else:
        idx_flatn = np.full(NWG * NB * NIDXB, -1, np.int16)
        idx_flatn[slot] = bankidx
        cnts = cnt.reshape(NWG, NB).astype(np.int32)
        empty = (cnts == 0)
        if empty.any():
            wg_e, b_e = np.nonzero(empty)
            idx_flatn[(wg_e * NB + b_e) * NIDXB] = 0
            cnts[empty] = 1
    idx_dma_n = idx_layout(idx_flatn)
    dst_flat = np.full(NWG * NB * NIDXB, -1.0, np.float32)
    dst_flat[slot] = (dst_g % P).astype(np.float32)
    dst_dma = np.ascontiguousarray(
        dst_flat.reshape(NWG, NB * Tb, P).transpose(0, 2, 1))

    iota_np = np.tile(np.arange(P, dtype=np.float32), (P, 1))
    ident_np = np.eye(P, dtype=np.float32)

    uids = np.asarray(user_ids).astype(np.int64)
    iids = np.asarray(item_ids).astype(np.int64) + NU

    in_maps = []
    for k in range(CORES):
        in_maps.append(dict(
            xT=xT,
            xTo=np.ascontiguousarray(xT[:, k * NCN:(k + 1) * NCN]),
            xo=np.ascontiguousarray(xpad[k * NCN:(k + 1) * NCN]),
            W1p=W1p, W2p=W2p, iota=iota_np, ident=ident_np,
            dstw=idx_dst(idx_dma_n, dst_dma, k, W)[1],
            idxw=idx_dst(idx_dma_n, dst_dma, k, W)[0],
            cnts=np.ascontiguousarray(cnts[k * W:(k + 1) * W]),
            cnts2=np.ascontiguousarray(cnts[k * W:(k + 1) * W]),
            uidx=uids[k * cfg["BB"]:(k + 1) * cfg["BB"]].astype(
                np.int32).reshape(P, BB // P),
            iidx=iids[k * cfg["BB"]:(k + 1) * cfg["BB"]].astype(
                np.int32).reshape(P, BB // P),
        ))
    return in_maps


def idx_dst(idx_dma, dst_dma, k, W):
    i = np.ascontiguousarray(
        idx_dma[k * W:(k + 1) * W].reshape(W * P, -1))
    d = np.ascontiguousarray(
        dst_dma[k * W:(k + 1) * W].reshape(W * P, -1))
    return i, d


DEFAULT_CFG = dict(
    CORES=8, W=148, NC_NODES=148 * P, NSTAR=8 * 148 * P, NTAB=NBANKS * BANK,
    NB=5, Tb=4, BB=2048, UN_D1=16, UN_D1B=4, UN_WIN=2, UN_D2=4,
)

_PROGRAM_CACHE = {}


def _get_program(cfg_key, cfg):
    if cfg_key not in _PROGRAM_CACHE:
        _PROGRAM_CACHE[cfg_key] = build_program(cfg)
    return _PROGRAM_CACHE[cfg_key]


def run(inputs, cfg=None, trace=False):
    cfg = dict(DEFAULT_CFG if cfg is None else cfg)
    # size Tb from the data (static program structure depends on it)
    src = np.asarray(inputs["edge_index"][0]).astype(np.int64)
    dst = np.asarray(inputs["edge_index"][1]).astype(np.int64)
    key = (dst // P) * cfg["NB"] + (src // BANK)
    cnt = np.bincount(key, minlength=cfg["CORES"] * cfg["W"] * cfg["NB"])
    cfg["Tb"] = max(int(np.ceil(cnt.max() / P)), 1)
    in_maps = prepare_inputs(cfg=cfg, **inputs)
    nc = _get_program(tuple(sorted(cfg.items())), cfg)
    res = run_bass_kernel_spmd(nc, in_maps,
                               core_ids=list(range(cfg["CORES"])),
                               trace=trace)
    outs = [res.results[k]["out"].reshape(-1) for k in range(cfg["CORES"])]
    return np.concatenate(outs).astype(np.float32), res


def kernel(user_table, item_table, W1, a1, W2, a2, edge_index, user_ids,
           item_ids):
    out, _ = run(dict(user_table=user_table, item_table=item_table, W1=W1,
                      a1=a1, W2=W2, a2=a2, edge_index=edge_index,
                      user_ids=user_ids, item_ids=item_ids))
    return out

